# revision 6
# baseline (speedup 1.0000x reference)
"""Trainium2 Bass kernel for nn_CircularConvolution_5403068858821.

The reference computes result[:, :, n] += 1 for m in range(M) -> a constant
tensor of shape [N, C, L_x + M - 1] filled with M (=16.0). The inputs are
never used arithmetically, so the optimal kernel is a pure HBM fill:
each of the 8 cores memsets an SBUF tile to 16.0 once and DMA-broadcasts
it over its shard of the output. No input bytes ever touch the device.

Sharding: data-parallel over batch N=32 -> 4 batches/core; per-core output
is [4*512, 4111] = [2048, 4111] f32 (~33.7 MB of HBM writes per core).
"""

import os

import numpy as np

import concourse.bass as bass
import concourse.mybir as mybir
from concourse.bass_utils import run_bass_kernel_spmd

# Problem constants (hardcoded per the grading contract).
N, C, L_X = 32, 512, 4096
M = 16
L = L_X + M - 1  # 4111
N_CORES = 8
N_SHARD = N // N_CORES  # 4 batches per core
ROWS = N_SHARD * C  # 2048 rows per core
FILL = float(M)

_CACHED_NC = None
LAST_RESULTS = None  # test harness introspection: last BassKernelResults


def _build_nc():
    """Emit the per-core Bass program: fill 2048*4111 f32 elements with 16.0.

    The shard is declared as one [128, 16*4111] DRAM tensor (the linear
    buffer reshapes to (4, 512, 4111) on the host; every element is the
    same constant so element order is irrelevant). A single [128, 4111]
    SBUF tile is memset once, then one dma_start with a stride-0
    (broadcast) source AP replicates it 16x across the free dim --
    ~33.7 MB of pure HBM writes, zero HBM reads, one DMA semaphore.
    """
    nc = bass.Bass()
    P = 128
    REPS = ROWS // P  # 16
    cols = REPS * L  # 65776
    out = nc.dram_tensor("out", [P, cols], mybir.dt.float32, kind="ExternalOutput")

    with (
        nc.Block() as block,
        nc.semaphore("vsem") as vsem,
        nc.semaphore("dma_sem") as dma_sem,
        nc.sbuf_tensor("src", [P, L], mybir.dt.float32) as src_t,
    ):
        src = (
            src_t[:]
            .rearrange("p (a w) -> p a w", a=1)
            .broadcast_to([P, REPS, L])
        )
        dst = out[:, :].rearrange("p (r w) -> p r w", r=REPS)

        @block.vector
        def _(v):
            v.memset(src_t[:], FILL).then_inc(vsem, 1)

        @block.sync
        def _(s):
            s.wait_ge(vsem, 1)
            s.dma_start(out=dst, in_=src).then_inc(dma_sem, 16)
            s.wait_ge(dma_sem, 16)

    return nc


def kernel(x: np.ndarray, complex_weight: np.ndarray) -> np.ndarray:
    global _CACHED_NC, LAST_RESULTS
    if _CACHED_NC is None:
        _CACHED_NC = _build_nc()

    core_ids = list(range(N_CORES))
    in_maps = [{} for _ in core_ids]
    res = run_bass_kernel_spmd(_CACHED_NC, in_maps, core_ids)
    LAST_RESULTS = res

    shards = [res.results[c]["out"].reshape(N_SHARD, C, L) for c in core_ids]
    out = np.concatenate(shards, axis=0)
    return np.ascontiguousarray(out, dtype=np.float32)


# revision 7
# speedup vs baseline: 1.0383x; 1.0383x over previous
"""Trainium2 Bass kernel for nn_CircularConvolution_5403068858821.

The reference computes result[:, :, n] += 1 for m in range(M) -> a constant
tensor of shape [N, C, L_x + M - 1] filled with M (=16.0). The inputs are
never used arithmetically, so the optimal kernel is a pure HBM fill:
each of the 8 cores memsets an SBUF tile to 16.0 once and DMA-broadcasts
it over its shard of the output. No input bytes ever touch the device.

Sharding: data-parallel over batch N=32 -> 4 batches/core; per-core output
is [4*512, 4111] = [2048, 4111] f32 (~33.7 MB of HBM writes per core).
"""

import os

import numpy as np

import concourse.bass as bass
import concourse.mybir as mybir
from concourse.bass_utils import run_bass_kernel_spmd

# Problem constants (hardcoded per the grading contract).
N, C, L_X = 32, 512, 4096
M = 16
L = L_X + M - 1  # 4111
N_CORES = 8
N_SHARD = N // N_CORES  # 4 batches per core
ROWS = N_SHARD * C  # 2048 rows per core
FILL = float(M)

_CACHED_NC = None
LAST_RESULTS = None  # test harness introspection: last BassKernelResults


def _build_nc():
    """Emit the per-core Bass program: fill 2048*4111 f32 elements with 16.0.

    The shard is declared as one [128, 16*4111] DRAM tensor (the linear
    buffer reshapes to (4, 512, 4111) on the host; every element is the
    same constant so element order is irrelevant). A single [128, 4111]
    SBUF tile is memset once, then one dma_start with a stride-0
    (broadcast) source AP replicates it 16x across the free dim --
    ~33.7 MB of pure HBM writes, zero HBM reads, one DMA semaphore.
    """
    nc = bass.Bass()
    P = 128
    cols = (ROWS // P) * L  # 65776 f32 per partition row
    W0 = 512  # memset width; bulk DMA replicates it via a stride-0 AP
    reps = cols // W0  # 128
    rem = cols - reps * W0  # 240 columns, second small DMA
    out = nc.dram_tensor("out", [P, cols], mybir.dt.float32, kind="ExternalOutput")

    with (
        nc.Block() as block,
        nc.semaphore("vsem") as vsem,
        nc.semaphore("dma_sem") as dma_sem,
        nc.sbuf_tensor("src", [P, W0], mybir.dt.float32) as src_t,
    ):
        src = (
            src_t[:]
            .rearrange("p (a w) -> p a w", a=1)
            .broadcast_to([P, reps, W0])
        )
        dst = out[:, : reps * W0].rearrange("p (r w) -> p r w", r=reps)

        @block.vector
        def _(v):
            v.memset(src_t[:], FILL).then_inc(vsem, 1)

        @block.sync
        def _(s):
            s.wait_ge(vsem, 1)
            s.dma_start(out=dst, in_=src).then_inc(dma_sem, 16)
            s.dma_start(out=out[:, reps * W0 :], in_=src_t[:, :rem]).then_inc(
                dma_sem, 16
            )
            s.wait_ge(dma_sem, 32)

    return nc


def kernel(x: np.ndarray, complex_weight: np.ndarray) -> np.ndarray:
    global _CACHED_NC, LAST_RESULTS
    if _CACHED_NC is None:
        _CACHED_NC = _build_nc()

    core_ids = list(range(N_CORES))
    in_maps = [{} for _ in core_ids]
    res = run_bass_kernel_spmd(_CACHED_NC, in_maps, core_ids)
    LAST_RESULTS = res

    shards = [res.results[c]["out"].reshape(N_SHARD, C, L) for c in core_ids]
    out = np.concatenate(shards, axis=0)
    return np.ascontiguousarray(out, dtype=np.float32)
